# revision 1
# baseline (speedup 1.0000x reference)
"""KAN layer kernel for 8 Trainium2 NeuronCores.

Math (reference):
    basis[b,i] = sum_h silu(x[b,i]*w1[i%K,h] + b1[i%K,h]) * w2[i%K,h] + b2[i%K]
    out[b,o]   = sum_i basis[b,i] * Wsum[o,i],   Wsum = W.sum(-1)   # [O,I]

Sharding: data-parallel over the input-feature axis I (16384 -> 8 x 2048).
Each core computes a partial out[64,1024] over its feature slice; host sums.

Per-core device program (memory-bound on reading its 42 MB W slice):
  - W arrives host-transposed as Wt[i,k,o]; the k-reduction happens *inside
    the DMA* via serial accum_op=add transfers (SDMA CCE), so Wsum[i,o]
    lands in SBUF with zero engine work and contraction (i) already on the
    partition axis -- no on-chip transposes anywhere.
  - basis is computed with i on partitions: ACT evaluates
    silu(w1*x+b1) with per-partition scale/bias vectors; DVE accumulates
    w2*silu(+b2) via fused scalar_tensor_tensor. Result acc[i,b] is directly
    the matmul lhsT.
  - 2 fp32 matmuls per i-tile accumulate into PSUM across all 16 i-tiles.
"""
import numpy as np

B, I, O, K, H = 64, 16384, 1024, 5, 16
NCORES = 8
IC = I // NCORES          # 2048 features per core
P = 128                   # partition tile
NT = IC // P              # 16 i-tiles per core
NB = B                    # 64
NO = O                    # 1024
PRW = 3 * H + 1           # packed param cols per i-tile: w1,b1,w2 (16 ea) + b2
CBW = NT * NB + NT * PRW  # const tile width: x block + param block

TRACE = False             # test.py sets True to capture an NTFF profile
LAST_RESULT = None


def _build():
    from contextlib import ExitStack
    from concourse import bacc, mybir, tile

    dt = mybir.dt.float32
    nc = bacc.Bacc("TRN2", target_bir_lowering=False, debug=False,
                   num_devices=NCORES)
    Wt = nc.declare_dram_parameter("Wt", [IC, K, NO], dt, isOutput=False)
    cbd = nc.declare_dram_parameter("cb", [P, CBW], dt, isOutput=False)
    out = nc.declare_dram_parameter("out", [NB, NO], dt, isOutput=True)

    with tile.TileContext(nc) as tc, ExitStack() as ctx:
        const = ctx.enter_context(tc.tile_pool(name="const", bufs=1))
        wpool = ctx.enter_context(tc.tile_pool(name="w", bufs=4))
        bpool = ctx.enter_context(tc.tile_pool(name="basis", bufs=3))
        spool = ctx.enter_context(tc.tile_pool(name="silu", bufs=3))
        opool = ctx.enter_context(tc.tile_pool(name="out", bufs=1))
        psum = ctx.enter_context(tc.tile_pool(name="psum", bufs=1, space="PSUM"))

        cb = const.tile([P, CBW], dt)
        nc.sync.dma_start(cb[:, :], cbd[:, :])

        ps0 = psum.tile([NB, 512], dt, tag="ps0")
        ps1 = psum.tile([NB, 512], dt, tag="ps1")

        for t in range(NT):
            # ---- Wsum[i,o] = sum_k Wt[i,k,o], reduced inside the DMA ----
            wsum = wpool.tile([P, NO], dt)
            nc.gpsimd.dma_start(wsum[:, :], Wt[t * P:(t + 1) * P, 0, :])
            for k in range(1, K):
                nc.gpsimd.dma_start(wsum[:, :], Wt[t * P:(t + 1) * P, k, :],
                                    accum_op=mybir.AluOpType.add)

            # ---- basisT[i,b] for this i-tile ----
            xs = cb[:, t * NB:(t + 1) * NB]
            pb = NT * NB + t * PRW
            acc = bpool.tile([P, NB], dt)
            for h in range(H):
                st = spool.tile([P, NB], dt)
                nc.scalar.activation(
                    st[:, :], xs, mybir.ActivationFunctionType.Silu,
                    bias=cb[:, pb + H + h:pb + H + h + 1],
                    scale=cb[:, pb + h:pb + h + 1],
                )
                if h == 0:
                    # acc = w2[:,0]*silu + b2
                    nc.vector.tensor_scalar(
                        acc[:, :], st[:, :],
                        cb[:, pb + 2 * H:pb + 2 * H + 1],
                        cb[:, pb + 3 * H:pb + 3 * H + 1],
                        op0=mybir.AluOpType.mult, op1=mybir.AluOpType.add,
                    )
                else:
                    # acc = w2[:,h]*silu + acc
                    nc.vector.scalar_tensor_tensor(
                        acc[:, :], st[:, :],
                        cb[:, pb + 2 * H + h:pb + 2 * H + h + 1],
                        acc[:, :],
                        op0=mybir.AluOpType.mult, op1=mybir.AluOpType.add,
                    )

            # ---- partial matmul: out[b,o] += basisT.T @ Wsum ----
            nc.tensor.matmul(ps0[:, :], acc[:, :], wsum[:, 0:512],
                             start=(t == 0), stop=(t == NT - 1))
            nc.tensor.matmul(ps1[:, :], acc[:, :], wsum[:, 512:1024],
                             start=(t == 0), stop=(t == NT - 1))

        out_sb = opool.tile([NB, NO], dt)
        nc.vector.tensor_copy(out_sb[:, 0:512], ps0[:, :])
        nc.vector.tensor_copy(out_sb[:, 512:1024], ps1[:, :])
        nc.sync.dma_start(out[:, :], out_sb[:, :])
    nc.compile()
    return nc


def kernel(x, w1, b1, w2, b2, W):
    global LAST_RESULT
    from concourse.bass_utils import run_bass_kernel_spmd

    x = np.asarray(x, dtype=np.float32)
    W = np.asarray(W, dtype=np.float32)
    w1 = np.asarray(w1, dtype=np.float32)
    b1 = np.asarray(b1, dtype=np.float32)
    w2 = np.asarray(w2, dtype=np.float32)
    b2 = np.asarray(b2, dtype=np.float32)

    # ---- host prep: W -> [I,K,O] (contraction-major layout for the PE) ----
    Wt_full = np.ascontiguousarray(W.reshape(O, I * K).T).reshape(I, K, O)

    idx = np.arange(I) % K
    w1e, b1e, w2e = w1[idx], b1[idx], w2[idx]          # [I,H]
    b2e = b2[idx][:, None]                             # [I,1]
    pr = np.concatenate([w1e, b1e, w2e, b2e], axis=1)  # [I, PRW]

    in_maps = []
    for c in range(NCORES):
        sl = slice(c * IC, (c + 1) * IC)
        # x slice, transposed to [i, b], then swizzled to SBUF layout [P, NT*NB]
        xt = np.ascontiguousarray(x[:, sl].T)          # [IC, NB]
        xt_sb = xt.reshape(NT, P, NB).transpose(1, 0, 2).reshape(P, NT * NB)
        pr_sb = pr[sl].reshape(NT, P, PRW).transpose(1, 0, 2).reshape(P, NT * PRW)
        cb = np.ascontiguousarray(
            np.concatenate([xt_sb, pr_sb], axis=1), dtype=np.float32)
        in_maps.append({"Wt": Wt_full[sl], "cb": cb})

    nc = _build()
    res = run_bass_kernel_spmd(nc, in_maps, list(range(NCORES)), trace=TRACE)
    LAST_RESULT = res
    out = np.zeros((B, O), dtype=np.float32)
    for c in range(NCORES):
        out += res.results[c]["out"]
    return out


# revision 5
# speedup vs baseline: 1.1394x; 1.1394x over previous
"""KAN layer kernel for 8 Trainium2 NeuronCores.

Math (reference):
    basis[b,i] = sum_h silu(x[b,i]*w1[i%K,h] + b1[i%K,h]) * w2[i%K,h] + b2[i%K]
    out[b,o]   = sum_i basis[b,i] * Wsum[o,i],   Wsum = W.sum(-1)   # [O,I]

Sharding: data-parallel over the input-feature axis I (16384 -> 8 x 2048).
Each core computes a partial out[64,1024] over its feature slice; host sums.

Per-core device program (memory-bound on reading its 42 MB W slice):
  - W arrives host-transposed as Wt[i,k,o]; the k-reduction happens *inside
    the DMA* via serial accum_op=add transfers (SDMA CCE), so Wsum[i,o]
    lands in SBUF with zero engine work and contraction (i) already on the
    partition axis -- no on-chip transposes anywhere.
  - basis is computed with i on partitions: ACT evaluates
    silu(w1*x+b1) with per-partition scale/bias vectors; DVE accumulates
    w2*silu(+b2) via fused scalar_tensor_tensor. Result acc[i,b] is directly
    the matmul lhsT.
  - 2 fp32 matmuls per i-tile accumulate into PSUM across all 16 i-tiles.
"""
import numpy as np

B, I, O, K, H = 64, 16384, 1024, 5, 16
NCORES = 8
IC = I // NCORES          # 2048 features per core
P = 128                   # partition tile
NT = IC // P              # 16 i-tiles per core
NB = B                    # 64
NO = O                    # 1024
PRW = 3 * H + 1           # packed param cols per i-tile: w1,b1,w2 (16 ea) + b2
CBW = NT * NB + NT * PRW  # const tile width: x block + param block

TRACE = False             # test.py sets True to capture an NTFF profile
LAST_RESULT = None


def _build():
    from contextlib import ExitStack
    from concourse import bacc, mybir, tile

    dt = mybir.dt.float32
    nc = bacc.Bacc("TRN2", target_bir_lowering=False, debug=False,
                   num_devices=NCORES)
    Wt = nc.declare_dram_parameter("Wt", [IC, K, NO], dt, isOutput=False)
    cbd = nc.declare_dram_parameter("cb", [P, CBW], dt, isOutput=False)
    out = nc.declare_dram_parameter("out", [NB, NO], dt, isOutput=True)

    with tile.TileContext(nc) as tc, ExitStack() as ctx:
        const = ctx.enter_context(tc.tile_pool(name="const", bufs=1))
        wpool = ctx.enter_context(tc.tile_pool(name="w", bufs=8))
        bpool = ctx.enter_context(tc.tile_pool(name="basis", bufs=16))
        spool = ctx.enter_context(tc.tile_pool(name="silu", bufs=3))
        opool = ctx.enter_context(tc.tile_pool(name="out", bufs=1))
        psum = ctx.enter_context(tc.tile_pool(name="psum", bufs=1, space="PSUM"))

        cb = const.tile([P, CBW], dt)
        nc.sync.dma_start(cb[:, :], cbd[:, :])

        ps0 = psum.tile([NB, 512], dt, tag="ps0")
        ps1 = psum.tile([NB, 512], dt, tag="ps1")

        # ---- basisT[i,b] for every i-tile (ACT/DVE only; no W dependency) ----
        accs = []
        for t in range(NT):
            xs = cb[:, t * NB:(t + 1) * NB]
            pb = NT * NB + t * PRW
            acc = bpool.tile([P, NB], dt)
            for h in range(H):
                st = spool.tile([P, NB], dt)
                nc.scalar.activation(
                    st[:, :], xs, mybir.ActivationFunctionType.Silu,
                    bias=cb[:, pb + H + h:pb + H + h + 1],
                    scale=cb[:, pb + h:pb + h + 1],
                )
                if h == 0:
                    # acc = w2[:,0]*silu + b2
                    nc.vector.tensor_scalar(
                        acc[:, :], st[:, :],
                        cb[:, pb + 2 * H:pb + 2 * H + 1],
                        cb[:, pb + 3 * H:pb + 3 * H + 1],
                        op0=mybir.AluOpType.mult, op1=mybir.AluOpType.add,
                    )
                else:
                    # acc = w2[:,h]*silu + acc
                    nc.vector.scalar_tensor_tensor(
                        acc[:, :], st[:, :],
                        cb[:, pb + 2 * H + h:pb + 2 * H + h + 1],
                        acc[:, :],
                        op0=mybir.AluOpType.mult, op1=mybir.AluOpType.add,
                    )
            accs.append(acc)

        # ---- Wsum[i,o] = sum_k Wt[i,k,o], reduced inside the DMA.
        # All SWDGE DMAs issue in program order from the one gpsimd
        # sequencer, and step k of a tile must wait for step k-1's
        # completion (~2us). Interleaving the chains of a window of tiles
        # keeps every wait pre-satisfied so the queue never stalls. ----
        WIN = 4
        wsums = [None] * NT
        for base in range(0, NT, WIN):
            grp = range(base, min(base + WIN, NT))
            for t in grp:
                wsums[t] = wpool.tile([P, NO], dt, tag="wsum", name=f"wsum{t}")
            for k in range(K):
                for t in grp:
                    nc.gpsimd.dma_start(
                        wsums[t][:, :], Wt[t * P:(t + 1) * P, k, :],
                        accum_op=(mybir.AluOpType.bypass if k == 0
                                  else mybir.AluOpType.add))

        # ---- partial matmuls: out[b,o] += basisT.T @ Wsum ----
        for t in range(NT):
            nc.tensor.matmul(ps0[:, :], accs[t][:, :], wsums[t][:, 0:512],
                             start=(t == 0), stop=(t == NT - 1))
            nc.tensor.matmul(ps1[:, :], accs[t][:, :], wsums[t][:, 512:1024],
                             start=(t == 0), stop=(t == NT - 1))

        out_sb = opool.tile([NB, NO], dt)
        nc.vector.tensor_copy(out_sb[:, 0:512], ps0[:, :])
        nc.vector.tensor_copy(out_sb[:, 512:1024], ps1[:, :])
        nc.sync.dma_start(out[:, :], out_sb[:, :])
    nc.compile()
    return nc


def kernel(x, w1, b1, w2, b2, W):
    global LAST_RESULT
    from concourse.bass_utils import run_bass_kernel_spmd

    x = np.asarray(x, dtype=np.float32)
    W = np.asarray(W, dtype=np.float32)
    w1 = np.asarray(w1, dtype=np.float32)
    b1 = np.asarray(b1, dtype=np.float32)
    w2 = np.asarray(w2, dtype=np.float32)
    b2 = np.asarray(b2, dtype=np.float32)

    # ---- host prep: W -> [I,K,O] (contraction-major layout for the PE) ----
    Wt_full = np.ascontiguousarray(W.reshape(O, I * K).T).reshape(I, K, O)

    idx = np.arange(I) % K
    w1e, b1e, w2e = w1[idx], b1[idx], w2[idx]          # [I,H]
    b2e = b2[idx][:, None]                             # [I,1]
    pr = np.concatenate([w1e, b1e, w2e, b2e], axis=1)  # [I, PRW]

    in_maps = []
    for c in range(NCORES):
        sl = slice(c * IC, (c + 1) * IC)
        # x slice, transposed to [i, b], then swizzled to SBUF layout [P, NT*NB]
        xt = np.ascontiguousarray(x[:, sl].T)          # [IC, NB]
        xt_sb = xt.reshape(NT, P, NB).transpose(1, 0, 2).reshape(P, NT * NB)
        pr_sb = pr[sl].reshape(NT, P, PRW).transpose(1, 0, 2).reshape(P, NT * PRW)
        cb = np.ascontiguousarray(
            np.concatenate([xt_sb, pr_sb], axis=1), dtype=np.float32)
        in_maps.append({"Wt": Wt_full[sl], "cb": cb})

    nc = _build()
    res = run_bass_kernel_spmd(nc, in_maps, list(range(NCORES)), trace=TRACE)
    LAST_RESULT = res
    out = np.zeros((B, O), dtype=np.float32)
    for c in range(NCORES):
        out += res.results[c]["out"]
    return out
